# revision 11
# baseline (speedup 1.0000x reference)
"""Bidirectional Mamba layer on 8 Trainium2 NeuronCores.

Sharding: data-parallel over batch (8 batches -> 8 cores). Each core runs
both directions (fwd on x, bwd on time-reversed x) for its batch.

Per-core algorithm (per direction), all in "d-major" layout [d on
partitions, time on free dim]:
  1. uzT = in_w @ x^T                (PE, bf16)
  2. causal depthwise conv + SiLU    (ACT scale-copy + 3 fused DVE STT)
  3. dblT = xp_w @ uc^T              (PE)  -> dt / B / C rows
  4. deltaT = softplus(dt_w @ dtT + dt_b)  (PE + ACT Softplus)
  5. per (d-chunk, s):  a = exp(A[d,s] * delta)   (ACT, per-partition scale)
                        b = (delta*uc) * bcast(B[s,:])  (DVE)
                        h = tensor_tensor_scan(a, b)    (DVE, fp32 state)
                        y += h * bcast(C[s,:])          (DVE)
  6. g = (uc*D + y) * silu(z); out = g^T @ out_w^T      (PE)
Host combines: out = out_f + reverse_time(out_b).
"""

import sys

sys.path.insert(0, "/opt/trn_rl_repo")

import numpy as np
import ml_dtypes

import concourse.bass as bass
import concourse.mybir as mybir
import bass_rust
from concourse import tile
from concourse.bass_utils import run_bass_kernel_spmd

BF16 = mybir.dt.bfloat16
F32 = mybir.dt.float32
AF = mybir.ActivationFunctionType
OP = mybir.AluOpType

D_MODEL = 512
D_INNER = 1024
D_STATE = 16
D_CONV = 4
DT_RANK = 32
BATCH = 8
SEQ = 1024

P = 128
NC_D = D_INNER // P  # 8 d-chunks
NC_T = SEQ // P      # 8 t-chunks
NN = SEQ // 512      # 2 psum-free chunks


def _dir_params(nc, d):
    """Declare per-direction dram parameters (host passes pre-transposed)."""
    return {
        "inwT": nc.declare_dram_parameter(f"inwT_{d}", [D_MODEL, 2 * D_INNER], BF16, isOutput=False),
        "xpwT": nc.declare_dram_parameter(f"xpwT_{d}", [D_INNER, DT_RANK + 2 * D_STATE], BF16, isOutput=False),
        "dtwT": nc.declare_dram_parameter(f"dtwT_{d}", [DT_RANK, D_INNER], BF16, isOutput=False),
        "outwT": nc.declare_dram_parameter(f"outwT_{d}", [D_INNER, D_MODEL], BF16, isOutput=False),
        "A": nc.declare_dram_parameter(f"A_{d}", [D_INNER, D_STATE], F32, isOutput=False),
        "convw": nc.declare_dram_parameter(f"convw_{d}", [D_INNER, D_CONV], F32, isOutput=False),
        "convb": nc.declare_dram_parameter(f"convb_{d}", [D_INNER, 1], F32, isOutput=False),
        "dtb": nc.declare_dram_parameter(f"dtb_{d}", [D_INNER, 1], F32, isOutput=False),
        "Dp": nc.declare_dram_parameter(f"Dp_{d}", [D_INNER, 1], F32, isOutput=False),
        "xT": nc.declare_dram_parameter(f"xT_{d}", [D_MODEL, SEQ], BF16, isOutput=False),
        "out": nc.declare_dram_parameter(f"out_{d}", [SEQ, D_MODEL], F32, isOutput=True),
        "oht": nc.declare_dram_parameter(f"oht_{d}", [2 * D_STATE, 2 * D_STATE * P], BF16, isOutput=False),
    }


def _one_direction(ctx_pools, tc, p):
    import contextlib

    nc = tc.nc

    cst = ctx_pools  # long-lived pool for this direction

    # ---- load weights ----
    inwT = [cst.tile([P, 2 * D_INNER], BF16, tag=f"inwT{k}", name=f"inwT{k}") for k in range(4)]
    for k in range(4):
        nc.sync.dma_start(inwT[k][:], p["inwT"][k * P:(k + 1) * P, :])
    xT = [cst.tile([P, SEQ], BF16, tag=f"xT{k}", name=f"xT{k}") for k in range(4)]
    for k in range(4):
        nc.sync.dma_start(xT[k][:], p["xT"][k * P:(k + 1) * P, :])
    xpwT = [cst.tile([P, 64], BF16, tag=f"xpwT{c}", name=f"xpwT{c}") for c in range(NC_D)]
    outwT = [cst.tile([P, D_MODEL], BF16, tag=f"outwT{c}", name=f"outwT{c}") for c in range(NC_D)]
    A_sb = [cst.tile([P, D_STATE], F32, tag=f"A{c}", name=f"A{c}") for c in range(NC_D)]
    convw = [cst.tile([P, D_CONV], F32, tag=f"convw{c}", name=f"convw{c}") for c in range(NC_D)]
    convb = [cst.tile([P, 1], F32, tag=f"convb{c}", name=f"convb{c}") for c in range(NC_D)]
    dtb = [cst.tile([P, 1], F32, tag=f"dtb{c}", name=f"dtb{c}") for c in range(NC_D)]
    Dp = [cst.tile([P, 1], F32, tag=f"Dp{c}", name=f"Dp{c}") for c in range(NC_D)]
    for c in range(NC_D):
        sl = slice(c * P, (c + 1) * P)
        nc.sync.dma_start(xpwT[c][:], p["xpwT"][sl, :])
        nc.sync.dma_start(outwT[c][:], p["outwT"][sl, :])
        nc.sync.dma_start(A_sb[c][:], p["A"][sl, :])
        nc.sync.dma_start(convw[c][:], p["convw"][sl, :])
        nc.sync.dma_start(convb[c][:], p["convb"][sl, :])
        nc.sync.dma_start(dtb[c][:], p["dtb"][sl, :])
        nc.sync.dma_start(Dp[c][:], p["Dp"][sl, :])
    dtwT = cst.tile([DT_RANK, D_INNER], BF16, tag="dtwT", name="dtwT")
    nc.sync.dma_start(dtwT[:], p["dtwT"][:])

    # persistent activations for this direction
    uT = [cst.tile([P, SEQ + D_CONV - 1], BF16, tag=f"uT{c}", name=f"uT{c}") for c in range(NC_D)]
    sz = [cst.tile([P, SEQ], BF16, tag=f"sz{c}", name=f"sz{c}") for c in range(NC_D)]
    ucT = [cst.tile([P, SEQ], BF16, tag=f"ucT{c}", name=f"ucT{c}") for c in range(NC_D)]
    delta = [cst.tile([P, SEQ], BF16, tag=f"delta{c}", name=f"delta{c}") for c in range(NC_D)]
    w_bf = [cst.tile([P, SEQ], BF16, tag=f"w{c}", name=f"w{c}") for c in range(NC_D)]
    y_sb = [cst.tile([P, SEQ], BF16, tag=f"y{c}", name=f"y{c}") for c in range(NC_D)]
    dt_bf = cst.tile([DT_RANK, SEQ], BF16, tag="dt_bf", name="dt_bf")
    bc_bf = cst.tile([2 * D_STATE, SEQ], BF16, tag="bc_bf", name="bc_bf")

    for c in range(NC_D):
        nc.vector.memset(uT[c][:, 0:D_CONV - 1], 0.0)
    tc.strict_bb_all_engine_barrier()

    with contextlib.ExitStack() as phase:
        ps1 = phase.enter_context(tc.tile_pool(name="ps1", bufs=4, space="PSUM"))
        # ---- GEMM1: uzT[m*128:(m+1)*128, :] ----
        for m in range(2 * NC_D):
            for n in range(NN):
                pt = ps1.tile([P, 512], F32, tag="g1", name="g1")
                for k in range(4):
                    nc.tensor.matmul(
                        pt[:],
                        inwT[k][:, m * P:(m + 1) * P],
                        xT[k][:, n * 512:(n + 1) * 512],
                        start=(k == 0),
                        stop=(k == 3),
                    )
                if m < NC_D:
                    nc.scalar.copy(
                        uT[m][:, D_CONV - 1 + n * 512: D_CONV - 1 + (n + 1) * 512],
                        pt[:],
                    )
                else:
                    nc.scalar.activation(
                        sz[m - NC_D][:, n * 512:(n + 1) * 512], pt[:], AF.Silu
                    )

        # ---- conv + SiLU ----
        t_pool = phase.enter_context(tc.tile_pool(name="conv_t", bufs=3))
        for c in range(NC_D):
            t0 = t_pool.tile([P, SEQ], BF16, tag="t0", name="t0")
            nc.vector.tensor_scalar_mul(t0[:], uT[c][:, 0:SEQ], convw[c][:, 0:1])
            for k in range(1, D_CONV):
                t1 = t_pool.tile([P, SEQ], BF16, tag="t0", name="t0")
                nc.vector.scalar_tensor_tensor(
                    t1[:], uT[c][:, k:k + SEQ], convw[c][:, k:k + 1], t0[:],
                    op0=OP.mult, op1=OP.add,
                )
                t0 = t1
            nc.scalar.activation(
                ucT[c][:], t0[:], AF.Silu, bias=convb[c][:, 0:1]
            )

    with contextlib.ExitStack() as phase:
        ps2 = phase.enter_context(tc.tile_pool(name="ps2", bufs=4, space="PSUM"))
        # ---- GEMM2: dblT [64, SEQ] ----
        for n in range(NN):
            pt = ps2.tile([64, 512], F32, tag="g2", name="g2")
            for c in range(NC_D):
                nc.tensor.matmul(
                    pt[:], xpwT[c][:], ucT[c][:, n * 512:(n + 1) * 512],
                    start=(c == 0), stop=(c == NC_D - 1),
                )
            nc.vector.tensor_copy(dt_bf[:, n * 512:(n + 1) * 512], pt[0:DT_RANK, :])
            nc.vector.tensor_copy(
                bc_bf[:, n * 512:(n + 1) * 512], pt[DT_RANK:64, :]
            )

        # ---- GEMM3: deltaT = softplus(dt_w @ dtT + dt_b) ----
        # softplus(x) = relu(x) + ln(1 + exp(-|x|))  (Softplus has no ACT table set)
        t_pool2 = phase.enter_context(tc.tile_pool(name="sp_t", bufs=3))
        for m in range(NC_D):
            for n in range(NN):
                pt = ps2.tile([P, 512], F32, tag="g3", name="g3")
                nc.tensor.matmul(
                    pt[:], dtwT[:, m * P:(m + 1) * P],
                    dt_bf[:, n * 512:(n + 1) * 512],
                    start=True, stop=True,
                )
                sl = slice(n * 512, (n + 1) * 512)
                ab = t_pool2.tile([P, 512], F32, tag="sp_ab", name="sp_ab")
                nc.scalar.activation(ab[:], pt[:], AF.Abs, bias=dtb[m][:, 0:1])
                en = t_pool2.tile([P, 512], F32, tag="sp_en", name="sp_en")
                nc.scalar.activation(en[:], ab[:], AF.Exp, scale=-1.0)
                l1 = t_pool2.tile([P, 512], F32, tag="sp_l1", name="sp_l1")
                nc.scalar.activation(l1[:], en[:], AF.Ln, bias=1.0)
                rl = t_pool2.tile([P, 512], F32, tag="sp_rl", name="sp_rl")
                nc.scalar.activation(rl[:], pt[:], AF.Relu, bias=dtb[m][:, 0:1])
                nc.vector.tensor_add(delta[m][:, sl], rl[:], l1[:])

        # ---- w = delta * uc ----
        for c in range(NC_D):
            nc.vector.tensor_mul(w_bf[c][:], delta[c][:], ucT[c][:])

    # ---- scan phase ----
    oht = cst.tile([2 * D_STATE, 2 * D_STATE * P], BF16, tag="oht", name="oht")
    nc.sync.dma_start(oht[:], p["oht"][:])
    with contextlib.ExitStack() as phase:
        bcp_pool = phase.enter_context(tc.tile_pool(name="bcp", bufs=4, space="PSUM"))
        bc_pool = phase.enter_context(tc.tile_pool(name="bc", bufs=3))
        ab_pool = phase.enter_context(tc.tile_pool(name="ab", bufs=3))
        h_pool = phase.enter_context(tc.tile_pool(name="h", bufs=3))
        for s in range(D_STATE):
            Bbc = bc_pool.tile([P, SEQ], BF16, tag="Bbc", name="Bbc")
            Cbc = bc_pool.tile([P, SEQ], BF16, tag="Cbc", name="Cbc")
            for src_row, dst in ((s, Bbc), (D_STATE + s, Cbc)):
                ps = bcp_pool.tile([P, SEQ], F32, tag="bcps", name="bcps")
                for n in range(NN):
                    nc.tensor.matmul(
                        ps[:, n * 512:(n + 1) * 512],
                        oht[:, src_row * P:(src_row + 1) * P],
                        bc_bf[:, n * 512:(n + 1) * 512],
                        start=True,
                        stop=True,
                    )
                nc.scalar.copy(dst[:], ps[:])
            for c in range(NC_D):
                a_t = ab_pool.tile([P, SEQ], BF16, tag="a", name="a")
                nc.scalar.activation(
                    a_t[:], delta[c][:], AF.Exp, scale=A_sb[c][:, s:s + 1]
                )
                b_t = ab_pool.tile([P, SEQ], BF16, tag="b", name="b")
                nc.vector.tensor_mul(b_t[:], w_bf[c][:], Bbc[:])
                h_t = h_pool.tile([P, SEQ], BF16, tag="h", name="h")
                nc.vector.tensor_tensor_scan(
                    h_t[:], a_t[:], b_t[:], 0.0, op0=OP.mult, op1=OP.add
                )
                if s == 0:
                    nc.vector.tensor_mul(y_sb[c][:], h_t[:], Cbc[:])
                else:
                    t_t = h_pool.tile([P, SEQ], BF16, tag="yt", name="yt")
                    nc.vector.tensor_mul(t_t[:], h_t[:], Cbc[:])
                    nc.vector.tensor_add(y_sb[c][:], y_sb[c][:], t_t[:])

    # ---- gate: y = (uc*D + y) * silu(z) ----
    for c in range(NC_D):
        nc.vector.scalar_tensor_tensor(
            y_sb[c][:], ucT[c][:], Dp[c][:, 0:1], y_sb[c][:],
            op0=OP.mult, op1=OP.add,
        )
        nc.vector.tensor_mul(y_sb[c][:], y_sb[c][:], sz[c][:])

    # ---- GEMM4: out[m*128:(m+1)*128, :] = g^T @ out_w^T ----
    with contextlib.ExitStack() as phase:
        ps4 = phase.enter_context(tc.tile_pool(name="ps4", bufs=3, space="PSUM"))
        o_pool = phase.enter_context(tc.tile_pool(name="o", bufs=3))
        for m in range(NC_T):
            pt = ps4.tile([P, D_MODEL], F32, tag="g4", name="g4")
            for c in range(NC_D):
                nc.tensor.matmul(
                    pt[:], y_sb[c][:, m * P:(m + 1) * P], outwT[c][:],
                    start=(c == 0), stop=(c == NC_D - 1),
                )
            ot = o_pool.tile([P, D_MODEL], F32, tag="ot", name="ot")
            nc.vector.tensor_copy(ot[:], pt[:])
            nc.sync.dma_start(p["out"][m * P:(m + 1) * P, :], ot[:])


def _split_excess_waits(nc):
    """walrus in this toolchain accepts at most one sync-wait per
    instruction (two for EventSemaphore); hoist the excess onto injected
    same-engine NoOps placed directly before the instruction."""
    for f in nc.m.functions:
        for bb in f.blocks:
            new_insts = []
            for inst in bb.instructions:
                si = inst.sync_info
                cap = 2 if isinstance(inst, mybir.InstEventSemaphore) else 1
                if si is not None and len(si.on_wait) > cap:
                    waits = list(si.on_wait)
                    for i, w in enumerate(waits[:-cap]):
                        nop = mybir.InstNoOp(
                            name=f"{inst.name}-wsplit{i}", ins=[], outs=[]
                        )
                        nop.engine = inst.engine
                        nop.sync_info = bass_rust.SyncInfo(on_wait=[w], on_update=[])
                        new_insts.append(nop)
                    inst.sync_info = bass_rust.SyncInfo(
                        on_wait=waits[-cap:], on_update=list(si.on_update)
                    )
                new_insts.append(inst)
            try:
                bb.instructions = new_insts
            except Exception:
                bb.instructions.clear()
                bb.instructions.extend(new_insts)


def build_bass():
    nc = bass.Bass()
    params = {d: _dir_params(nc, d) for d in ("f", "b")}
    with tile.TileContext(nc) as tc:
        for d in ("f", "b"):
            with tc.tile_pool(name=f"cst_{d}", bufs=1) as cst:
                _one_direction(cst, tc, params[d])
    _split_excess_waits(nc)
    return nc


def _prep_dir(w):
    """Host-side prep of one direction's weights -> dram param arrays."""
    bf = ml_dtypes.bfloat16
    in_w, conv_w, conv_b, xp_w, dt_w, dt_b, A_log, Dp, out_w = w
    return {
        "inwT": np.ascontiguousarray(in_w.T).astype(bf),
        "xpwT": np.ascontiguousarray(xp_w.T).astype(bf),
        "dtwT": np.ascontiguousarray(dt_w.T).astype(bf),
        "outwT": np.ascontiguousarray(out_w.T).astype(bf),
        "A": np.ascontiguousarray(-np.exp(A_log.astype(np.float64))).astype(np.float32),
        "convw": np.ascontiguousarray(conv_w).astype(np.float32),
        "convb": np.ascontiguousarray(conv_b).reshape(D_INNER, 1).astype(np.float32),
        "dtb": np.ascontiguousarray(dt_b).reshape(D_INNER, 1).astype(np.float32),
        "Dp": np.ascontiguousarray(Dp).reshape(D_INNER, 1).astype(np.float32),
        "oht": np.kron(np.eye(2 * D_STATE, dtype=np.float32), np.ones((1, P), np.float32)).astype(bf),
    }


_CACHED = {}


def kernel(
    x,
    in_w_f, conv_w_f, conv_b_f, xp_w_f, dt_w_f, dt_b_f, A_log_f, D_f, out_w_f,
    in_w_b, conv_w_b, conv_b_b, xp_w_b, dt_w_b, dt_b_b, A_log_b, D_b, out_w_b,
):
    bf = ml_dtypes.bfloat16
    x = np.asarray(x, dtype=np.float32)

    if "nc" not in _CACHED:
        _CACHED["nc"] = build_bass()
    nc = _CACHED["nc"]

    wf = _prep_dir((in_w_f, conv_w_f, conv_b_f, xp_w_f, dt_w_f, dt_b_f,
                    A_log_f, D_f, out_w_f))
    wb = _prep_dir((in_w_b, conv_w_b, conv_b_b, xp_w_b, dt_w_b, dt_b_b,
                    A_log_b, D_b, out_w_b))

    in_maps = []
    for b in range(BATCH):
        m = {}
        for d, wd in (("f", wf), ("b", wb)):
            for k, v in wd.items():
                m[f"{k}_{d}"] = v
        m["xT_f"] = np.ascontiguousarray(x[b].T).astype(bf)
        m["xT_b"] = np.ascontiguousarray(x[b][::-1].T).astype(bf)
        in_maps.append(m)

    res = run_bass_kernel_spmd(nc, in_maps, core_ids=list(range(BATCH)))
    out = np.empty((BATCH, SEQ, D_MODEL), np.float32)
    for b in range(BATCH):
        rb = res.results[b]
        out[b] = rb["out_f"] + rb["out_b"][::-1]
    return out
